# revision 1
# baseline (speedup 1.0000x reference)
"""ChildSum TreeLSTM (B=64 trees, N=512 nodes, D=300) on 8 NeuronCores.

Strategy: data-parallel over trees (8 trees/core). Within a core, nodes are
level-scheduled by height ("waves"); nodes are packed wave-major (sorted by
parent position within each wave) into 128-slot chunks, so child-sum
aggregation becomes small dense matmuls against host-built one-hot selection
blocks.  All matmul traffic is bf16 (PSUM accumulation in fp32); the wide
windows (~128 nodes) run node-on-partition ("N mode"), while the small
late-wave windows run feature-on-partition ("T mode") so their matmul cost
scales with the actual node count instead of the 300/900-wide gate outputs.
The transposed x / parent-x blocks for every window are resident in SBUF
(one bulk DMA), which removes the per-window descriptor storm of the
column-sliced loads.
"""

import hashlib
import numpy as np
import ml_dtypes

BF16 = ml_dtypes.bfloat16

D = 300
DC = 100          # d-chunk (3 chunks of 100 partitions)
NCORES = 8
P = 128
T_THRESH = 100    # windows narrower than this run in transposed (T) mode


# ----------------------------------------------------------------- schedule

class _Sched:
    pass


def _build_schedule(parent):
    """parent: [B, N] int array, parent[b,t] in (t, N]; N = sentinel."""
    B, N = parent.shape
    tpc = B // NCORES

    heights = np.zeros((B, N), np.int32)
    for b in range(B):
        h = np.zeros(N + 1, np.int32)
        pb = parent[b]
        for t in range(N):
            ht = h[t] + 1
            p = pb[t]
            if ht > h[p]:
                h[p] = ht
        heights[b] = h[:N]

    Hs = [int(heights[c * tpc:(c + 1) * tpc].max()) + 1 for c in range(NCORES)]
    H = max(Hs)

    sizes = np.zeros((NCORES, H), np.int64)
    for c in range(NCORES):
        cnt = np.bincount(heights[c * tpc:(c + 1) * tpc].ravel(), minlength=H)
        sizes[c] = cnt
    env_real = sizes.max(0)                     # real envelope size per wave
    c_env = ((env_real + P - 1) // P) * P       # 128-padded for ST addressing
    off = np.zeros(H + 1, np.int64)
    off[1:] = np.cumsum(c_env)
    P_total = int(off[H])
    NCH = (P_total + P - 1) // P

    # per-core packing: waves descending so parent positions exist first
    pos_all = np.full((NCORES, tpc, N), -1, np.int64)
    BIG = np.iinfo(np.int64).max
    for c in range(NCORES):
        w = heights[c * tpc:(c + 1) * tpc]
        pb = parent[c * tpc:(c + 1) * tpc]
        pos = pos_all[c]
        for v in range(H - 1, -1, -1):
            bs, ts = np.nonzero(w == v)
            if len(bs) == 0:
                continue
            pp = np.empty(len(bs), np.int64)
            for i in range(len(bs)):
                p = pb[bs[i], ts[i]]
                pp[i] = pos[bs[i], p] if p < N else BIG
            order = np.argsort(pp, kind="stable")
            pos[bs[order], ts[order]] = off[v] + np.arange(len(bs))

    # parent packed position per packed slot (-1 = sentinel parent or padding)
    parr = np.full((NCORES, NCH * P), -1, np.int64)
    for c in range(NCORES):
        pb = parent[c * tpc:(c + 1) * tpc]
        pos = pos_all[c]
        for b in range(tpc):
            for t in range(N):
                p = pb[b, t]
                parr[c, pos[b, t]] = pos[b, p] if p < N else -1

    # windows: one per 128-chunk; wl = envelope-real width (<= 128)
    windows = []  # (v, start, wl, mode)
    for v in range(H):
        s = int(off[v])
        rem = int(env_real[v])
        while rem > 0:
            wl = min(P, rem)
            mode = "N"
            windows.append((v, s, wl, mode))
            s += P
            rem -= wl

    # selection blocks per window: ST chunks containing any child (any core)
    blocks_by_window = []  # list of list of (global_block_idx, kc)
    block_defs = []        # (win_idx, kc, s, wl)
    for wi, (v, s, wl, mode) in enumerate(windows):
        blks = []
        if v > 0:
            chunks = set()
            for c in range(NCORES):
                childpos = np.nonzero((parr[c] >= s) & (parr[c] < s + wl))[0]
                chunks.update((childpos // P).tolist())
            for kc in sorted(chunks):
                blks.append((len(block_defs), kc))
                block_defs.append((wi, kc, s, wl))
        blocks_by_window.append(blks)

    # ---- tail staging: children of parents in waves >= STAGE_V are copied
    # (at produce time) into compact staging chunks, so tail windows need
    # only a few selection blocks instead of one per source chunk.
    STAGE_V = 99  # staging disabled: union-envelope made it a net loss
    LAG = 2   # stage only children consumed >= LAG waves later (hides DMA)
    wave_of = np.zeros(NCH * P, np.int64)
    for v in range(H):
        wave_of[off[v]:off[v + 1]] = v
    stage_src = []   # (wi, ch, a_local, length, cum)
    cum = 0
    for wi, (v, s, wl, mode) in enumerate(windows):
        tv = max(STAGE_V, v + LAG)
        thr = int(off[tv]) if tv < H else 1 << 60
        a = None
        for c in range(NCORES):
            idx = np.nonzero(parr[c, s:s + wl] >= thr)[0]
            if len(idx):
                a = idx[0] if a is None else min(a, int(idx[0]))
        if a is None:
            continue
        ln = wl - int(a)
        stage_src.append((wi, s // P, int(a), ln, cum))
        cum += ln
    NSROW = cum
    NSC = max(1, (NSROW + P - 1) // P)
    staged_pos = np.full(NCH * P, -1, np.int64)
    for (wi, ch, a, ln, cm) in stage_src:
        s = windows[wi][1]
        staged_pos[s + a:s + a + ln] = cm + np.arange(ln)
    # per-window staging DMA segments, split at 128-boundaries:
    # wi -> list of (src_lo, dst_sc, dst_lo, length)
    stage_dma = {}
    for (wi, ch, a, ln, cm) in stage_src:
        segs = []
        done = 0
        while done < ln:
            q = cm + done
            sc_i, lo = q // P, q % P
            take = min(ln - done, P - lo)
            segs.append((a + done, sc_i, lo, take))
            done += take
        stage_dma[wi] = segs

    # rebuild blocks for tail windows: staged chunks for old children,
    # direct chunks for recent ones (typed ('S'|'D', chunk))
    def _use_staged(p, v):
        return staged_pos[p] >= 0 and wave_of[p] + LAG <= v

    for wi, (v, s, wl, mode) in enumerate(windows):
        if v < STAGE_V:
            continue
        chunks = set()
        for c in range(NCORES):
            childpos = np.nonzero((parr[c] >= s) & (parr[c] < s + wl))[0]
            for p in childpos:
                if _use_staged(p, v):
                    chunks.add(("S", int(staged_pos[p] // P)))
                else:
                    chunks.add(("D", int(p // P)))
        blks = []
        for kc in sorted(chunks):
            blks.append((len(block_defs), kc))
            block_defs.append((wi, kc, s, wl))
        blocks_by_window[wi] = blks
    # early windows keep direct chunks; normalize their block keys to typed
    for wi, (v, s, wl, mode) in enumerate(windows):
        if v < STAGE_V:
            blocks_by_window[wi] = [(gbi, ("D", int(kc)))
                                    for (gbi, kc) in blocks_by_window[wi]]
    sc_use_staged = _use_staged

    sc = _Sched()
    sc.B, sc.N, sc.tpc, sc.H = B, N, tpc, H
    sc.STAGE_V, sc.NSC, sc.NSROW = STAGE_V, NSC, NSROW
    sc.staged_pos, sc.stage_dma = staged_pos, stage_dma
    sc.LAG, sc.wave_of, sc.use_staged = LAG, wave_of, sc_use_staged
    sc.env_real, sc.c_env, sc.off = env_real, c_env, off
    sc.P_total, sc.NCH = P_total, NCH
    sc.pos_all, sc.parr = pos_all, parr
    sc.windows = windows
    sc.blocks_by_window = blocks_by_window
    sc.block_defs = block_defs
    sc.MAXBLK = max(1, max((len(b) for b in blocks_by_window), default=1))
    # flat offsets of each window's block run in the packed sel stream
    sc.selw_off = {}
    run = 0
    for wi, blks in enumerate(blocks_by_window):
        sc.selw_off[wi] = run
        run += len(blks)
    sc.NB = max(1, run)
    return sc


def _build_core_inputs(sc, c, embs, parent):
    """Per-core input arrays (weights are shared, added separately)."""
    tpc, N, NCH = sc.tpc, sc.N, sc.NCH
    pos = sc.pos_all[c]
    pa = NCH * P

    # packed node -> (b_local, t)
    node_b = np.full(pa, -1, np.int64)
    node_t = np.full(pa, -1, np.int64)
    bs, ts = np.nonzero(pos >= 0)
    node_b[pos[bs, ts]] = bs
    node_t[pos[bs, ts]] = ts

    emb_c = embs[c * tpc:(c + 1) * tpc]  # [tpc, N, D]
    x_rows = np.zeros((pa, D), np.float32)
    real = node_b >= 0
    x_rows[real] = emb_c[node_b[real], node_t[real]]

    pb = parent[c * tpc:(c + 1) * tpc]
    xp_rows = np.zeros((pa, D), np.float32)
    pvals = np.where(real, pb[np.maximum(node_b, 0), np.maximum(node_t, 0)], N)
    has_par = real & (pvals < N)
    xp_rows[has_par] = emb_c[node_b[has_par], pvals[has_par]]

    # per-window input block: transposed x / transposed xp / node-major x
    # rows, all bf16 in one [NCH, 128, 1068] tensor (one DMA per window).
    # Partition dim 128: DGE spreads descriptors of 128-partition DMAs
    # round-robin across all 16 queues, others pin to queue 0.
    xxp = np.zeros((NCH, P, 2 * 3 * P + D), BF16)
    xv = xxp[:, :, :2 * 3 * P].reshape(NCH, P, 2, 3, P)
    for wi, (v, s, wl, mode) in enumerate(sc.windows):
        ch = s // P
        xb = x_rows[s:s + wl].astype(BF16)
        xpb = xp_rows[s:s + wl].astype(BF16)
        for r in range(3):
            xv[ch, :DC, 0, r, :wl] = xb[:, r * DC:(r + 1) * DC].T
            xv[ch, :DC, 1, r, :wl] = xpb[:, r * DC:(r + 1) * DC].T
        xv[ch, DC, 0, 2, :wl] = 1.0
        xv[ch, DC, 1, 2, :wl] = 1.0
        xxp[ch, :wl, 2 * 3 * P:] = xb

    # selection blocks, packed per window in SBUF image order:
    # window run of nblk blocks stored as [128 rows, nblk, 128 cols]
    sel = np.zeros((sc.NB, P, P), BF16)
    parr_c = sc.parr[c]
    for wi, blks in enumerate(sc.blocks_by_window):
        if not blks:
            continue
        nblk = len(blks)
        v, s, wl, mode = sc.windows[wi]
        arr = np.zeros((P, nblk, P), BF16)
        kc2bi = {kc: bi for bi, (gbi, kc) in enumerate(blks)}
        childpos = np.nonzero((parr_c >= s) & (parr_c < s + wl))[0]
        for p in childpos:
            if v >= sc.STAGE_V and sc.use_staged(p, v):
                q = int(sc.staged_pos[p])
                arr[q % P, kc2bi[("S", q // P)], parr_c[p] - s] = 1.0
            else:
                arr[int(p % P), kc2bi[("D", int(p // P))], parr_c[p] - s] = 1.0
        o = sc.selw_off[wi]
        sel[o:o + nblk] = arr.reshape(nblk, P, P)

    return {
        "xxp": xxp,
        "sel": sel,
    }


def _shared_weights(Wx, bx, Wh, bh, Wt, bt):
    def chunked_x(Wmat, bias):
        # Wmat: [300, M] -> [128, 3, M] with bias row in chunk 2 (partition
        # dim padded to 128 so the load spreads across DMA queues)
        M = Wmat.shape[1]
        out = np.zeros((P, 3, M), np.float32)
        for r in range(3):
            out[:DC, r] = Wmat[r * DC:(r + 1) * DC]
        out[DC, 2] = bias
        return out.astype(BF16)

    def chunked_h(Wmat):
        M = Wmat.shape[1]
        out = np.zeros((P, 3, M), np.float32)
        for r in range(3):
            out[:DC, r] = Wmat[r * DC:(r + 1) * DC]
        return out.astype(BF16)

    wx_iou = np.concatenate([Wx[0], Wx[1], Wx[2]], axis=1)  # [300, 900]
    wh_iou = np.concatenate([Wh[0], Wh[1], Wh[2]], axis=1)
    b_iou = np.concatenate([bx[0] + bh[0], bx[1] + bh[1], bx[2] + bh[2]])
    return {
        "wioux": chunked_x(wx_iou, b_iou),
        "wiouh": chunked_h(wh_iou),
        "wfx": chunked_x(Wx[3], bx[3] + bh[3]),
        "wfh": chunked_h(Wh[3]),
        "wtt": chunked_x(Wt, bt),
    }


# -------------------------------------------------------------- bass module

def _build_bass(sc):
    import concourse.mybir as mybir
    import concourse.tile as tile
    from concourse import bacc
    from concourse.masks import make_identity

    f32 = mybir.dt.float32
    bf16 = mybir.dt.bfloat16
    AF = mybir.ActivationFunctionType
    OP = mybir.AluOpType

    NCH, NB, H = sc.NCH, sc.NB, sc.H
    MAXBLK = sc.MAXBLK

    nc = bacc.Bacc()
    xxp_d = nc.dram_tensor("xxp", [NCH, P, 2 * 3 * P + D], bf16,
                           kind="ExternalInput")
    sel_d = nc.dram_tensor("sel", [NB, P, P], bf16, kind="ExternalInput")
    wioux_d = nc.dram_tensor("wioux", [P, 3, 3 * D], bf16, kind="ExternalInput")
    wiouh_d = nc.dram_tensor("wiouh", [P, 3, 3 * D], bf16, kind="ExternalInput")
    wfx_d = nc.dram_tensor("wfx", [P, 3, D], bf16, kind="ExternalInput")
    wfh_d = nc.dram_tensor("wfh", [P, 3, D], bf16, kind="ExternalInput")
    wtt_d = nc.dram_tensor("wtt", [P, 3, D], bf16, kind="ExternalInput")
    out_d = nc.dram_tensor("out", [NCH, P, D], bf16, kind="ExternalOutput")

    with tile.TileContext(nc) as tc:
        with (
            tc.tile_pool(name="const", bufs=1) as constp,
            tc.tile_pool(name="stp", bufs=1) as stp,
            tc.tile_pool(name="stream", bufs=6) as streamp,
            tc.tile_pool(name="ew", bufs=4) as ewp,
            tc.tile_pool(name="ps", bufs=1, space="PSUM") as psp,
        ):
            ident = constp.tile([P, P], bf16)
            make_identity(nc, ident[:])

            wioux = constp.tile([P, 3, 3 * D], bf16)
            nc.sync.dma_start(wioux[:], wioux_d[:])
            wiouh = constp.tile([P, 3, 3 * D], bf16)
            nc.sync.dma_start(wiouh[:], wiouh_d[:])
            wfx = constp.tile([P, 3, D], bf16)
            nc.sync.dma_start(wfx[:], wfx_d[:])
            wfh = constp.tile([P, 3, D], bf16)
            nc.sync.dma_start(wfh[:], wfh_d[:])
            wtt = constp.tile([P, 3, D], bf16)
            nc.sync.dma_start(wtt[:], wtt_d[:])
            # resident packed state, one tile per 128-slot chunk:
            # [128 slots, 6, 100] = st(300) | fst(300)
            STc = [stp.tile([P, 6, DC], bf16, name=f"stc{ch}", tag=f"stc{ch}")
                   for ch in range(NCH)]
            for ch in range(NCH):
                nc.gpsimd.memset(STc[ch][:], 0.0)
            SG = [stp.tile([P, 6, DC], bf16, name=f"sg{i}", tag=f"sg{i}")
                  for i in range(sc.NSC)]
            for i in range(sc.NSC):
                nc.gpsimd.memset(SG[i][:], 0.0)

            # PSUM banks: z*/g/f/fc flat [128, 384]; hs/tp shaped [128, 3, 128]
            def ptf(tag):
                return psp.tile([P, 3 * P], f32, tag=tag, name=tag)

            def pt3(tag, dt=f32):
                return psp.tile([P, 3, P], dt, tag=tag, name=tag)

            for wi, (v, s, wl, mode) in enumerate(sc.windows):
                ch = s // P
                blks = sc.blocks_by_window[wi]
                nblk = len(blks)
                last_wave = (v == H - 1)

                xw = streamp.tile([P, 2 * 3 * P + D], bf16, tag="xw")
                nc.sync.dma_start(xw[:], xxp_d[ch])

                if v > 0:
                    def _src(kc):
                        return SG[kc[1]] if kc[0] == "S" else STc[kc[1]]
                    selt = streamp.tile([P, MAXBLK, P], bf16, tag="sel")
                    o = sc.selw_off[wi]
                    nc.sync.dma_start(selt[:, 0:nblk, :], sel_d[o:o + nblk])
                    hs = pt3("hs")
                    fc = ptf("fc")
                    # hsumT[f, p] = sum_child st[child, f]
                    for r in range(3):
                        for bi, (gbi, kc) in enumerate(blks):
                            nc.tensor.matmul(
                                hs[0:DC, r, :wl],
                                lhsT=_src(kc)[:, r, :],
                                rhs=selt[:, bi, :wl],
                                start=(bi == 0), stop=(bi == nblk - 1))
                    hsumT = ewp.tile([DC, 3, P], bf16, tag="hsumT")
                    nc.vector.tensor_copy(hsumT[:, :, :wl], hs[0:DC, 0:3, :wl])
                    # fc[p, f] = sum_child f*st
                    for bi, (gbi, kc) in enumerate(blks):
                        nc.tensor.matmul(
                            fc[:wl, 0:D],
                            lhsT=selt[:, bi, :wl],
                            rhs=_src(kc)[:, 3:6, :],
                            start=(bi == 0), stop=(bi == nblk - 1))

                z = [ptf("z0"), ptf("z1"), ptf("z2")]
                g_ps = ptf("g")

                # iou pre-activations: [wl, 300] per gate
                for k in range(3):
                    for gi in range(3):
                        nc.tensor.matmul(
                            z[gi][:wl, 0:D],
                            lhsT=xw[0:DC + 1, k * P:k * P + wl],
                            rhs=wioux[0:DC + 1, k, gi * D:(gi + 1) * D],
                            start=(k == 0), stop=(v == 0 and k == 2))
                if v > 0:
                    for k in range(3):
                        for gi in range(3):
                            nc.tensor.matmul(
                                z[gi][:wl, 0:D],
                                lhsT=hsumT[:, k, :wl],
                                rhs=wiouh[0:DC, k, gi * D:(gi + 1) * D],
                                start=False, stop=(k == 2))
                # highway gate: g = tanh(x @ Wt + bt)
                for k in range(3):
                    nc.tensor.matmul(
                        g_ps[:wl, 0:D], lhsT=xw[0:DC + 1, k * P:k * P + wl],
                        rhs=wtt[0:DC + 1, k, :],
                        start=(k == 0), stop=(k == 2))

                g_sb = ewp.tile([P, D], bf16, tag="g_sb")
                nc.scalar.activation(g_sb[:wl], g_ps[:wl, 0:D], AF.Tanh)
                i_sb = ewp.tile([P, D], bf16, tag="i_sb")
                nc.scalar.activation(i_sb[:wl], z[0][:wl, 0:D], AF.Sigmoid)
                o_sb = ewp.tile([P, D], bf16, tag="o_sb")
                nc.scalar.activation(o_sb[:wl], z[1][:wl, 0:D], AF.Sigmoid)
                u_sb = ewp.tile([P, D], bf16, tag="u_sb")
                nc.scalar.activation(u_sb[:wl], z[2][:wl, 0:D], AF.Tanh)

                c_sb = ewp.tile([P, D], f32, tag="c_sb")
                nc.vector.tensor_tensor(c_sb[:wl], i_sb[:wl], u_sb[:wl],
                                        OP.mult)
                if v > 0:
                    nc.vector.tensor_tensor(c_sb[:wl], c_sb[:wl],
                                            fc[:wl, 0:D], OP.add)
                tc_sb = ewp.tile([P, D], bf16, tag="tc_sb")
                nc.scalar.activation(tc_sb[:wl], c_sb[:wl], AF.Tanh)
                h_sb = ewp.tile([P, D], bf16, tag="h_sb")
                nc.vector.tensor_tensor(h_sb[:wl], o_sb[:wl], tc_sb[:wl],
                                        OP.mult)
                # st = x + (h - x) * g
                d_sb = ewp.tile([P, D], bf16, tag="d_sb")
                nc.vector.tensor_tensor(d_sb[:wl], h_sb[:wl],
                                        xw[:wl, 6 * P:6 * P + D], OP.subtract)
                dg_sb = ewp.tile([P, D], bf16, tag="dg_sb")
                nc.vector.tensor_tensor(dg_sb[:wl], d_sb[:wl], g_sb[:wl],
                                        OP.mult)
                nc.vector.tensor_tensor(STc[ch][:wl, 0:3, :], dg_sb[:wl],
                                        xw[:wl, 6 * P:6 * P + D], OP.add)
                nc.scalar.dma_start(out_d[ch], STc[ch][:, 0:3, :])

                if last_wave:
                    continue

                # stT for the f-gate hidden-side matmul
                tp = pt3("tp", bf16)
                for r in range(3):
                    nc.tensor.transpose(tp[0:DC, r, :wl],
                                        STc[ch][:wl, r, :],
                                        ident[:wl, :wl])
                stT = ewp.tile([DC, 3, P], bf16, tag="stT")
                nc.vector.tensor_copy(stT[:, :, :wl], tp[0:DC, 0:3, :wl])

                # f = sigmoid(xp @ Wxf + st @ Whf + b); fst = f * st
                f_ps = ptf("f")
                for k in range(3):
                    nc.tensor.matmul(
                        f_ps[:wl, 0:D], lhsT=xw[0:DC + 1, (3 + k) * P:(3 + k) * P + wl],
                        rhs=wfx[0:DC + 1, k, :],
                        start=(k == 0), stop=False)
                for k in range(3):
                    nc.tensor.matmul(
                        f_ps[:wl, 0:D], lhsT=stT[:, k, :wl],
                        rhs=wfh[0:DC, k, :],
                        start=False, stop=(k == 2))
                f_sb = ewp.tile([P, D], bf16, tag="f_sb")
                nc.scalar.activation(f_sb[:wl], f_ps[:wl, 0:D], AF.Sigmoid)
                nc.vector.tensor_tensor(STc[ch][:wl, 3:6, :], f_sb[:wl],
                                        STc[ch][:wl, 0:3, :], OP.mult)

                # stage the suffix rows whose parents are in the tail waves
                for (src_lo, sc_i, dst_lo, take) in sc.stage_dma.get(wi, ()):
                    nc.sync.dma_start(
                        SG[sc_i][dst_lo:dst_lo + take, :, :],
                        STc[ch][src_lo:src_lo + take, :, :])


    nc.compile()
    return nc


# ------------------------------------------------------------------- driver

_CACHE = {}
LAST_RESULT = None


def kernel(embs, Wx, bx, Wh, bh, Wt, bt, parent):
    global LAST_RESULT
    embs = np.asarray(embs, np.float32)
    Wx = np.asarray(Wx, np.float32)
    bx = np.asarray(bx, np.float32)
    Wh = np.asarray(Wh, np.float32)
    bh = np.asarray(bh, np.float32)
    Wt = np.asarray(Wt, np.float32)
    bt = np.asarray(bt, np.float32)
    parent = np.asarray(parent, np.int64)

    key = hashlib.sha256(parent.tobytes()).hexdigest()
    if key in _CACHE:
        sc, nc = _CACHE[key]
    else:
        sc = _build_schedule(parent)
        nc = _build_bass(sc)
        _CACHE[key] = (sc, nc)

    wts = _shared_weights(Wx, bx, Wh, bh, Wt, bt)
    in_maps = []
    for c in range(NCORES):
        m = _build_core_inputs(sc, c, embs, parent)
        m.update(wts)
        in_maps.append(m)

    from concourse.bass_utils import run_bass_kernel_spmd
    res = run_bass_kernel_spmd(nc, in_maps, core_ids=list(range(NCORES)))
    LAST_RESULT = res

    B, N = parent.shape
    tpc = B // NCORES
    S = np.zeros((B, N, D), np.float32)
    for c in range(NCORES):
        flat = np.asarray(res.results[c]["out"]).astype(np.float32)
        flat = flat.reshape(sc.NCH * P, D)
        pos = sc.pos_all[c]
        S[c * tpc:(c + 1) * tpc] = flat[pos.reshape(-1)].reshape(tpc, N, D)
    return S

